# revision 21
# baseline (speedup 1.0000x reference)
"""BinaryTreeRNN Trainium2 kernel — 8-core data-parallel, v2.9.

Contract: kernel(**inputs) takes FULL unsharded inputs (x [4M,16] f32 plus tiny
tree params) and returns the FULL [4M] f32 output.

Design (per core, N_core = 500k samples, padded to 501760 = 560 blocks x 896):
  * Host folds all tree parameters in float64: softmax(om) -> per-node
    (A, S, C, P); S*sin(s)+C*cos(s) == R*sin(s+phi); parent phases are pushed
    into children's outputs with exact algebraic compensation.
  * x ships as PURE fp16 (~1.5e-3 L2 rel err vs the 2e-2 gate), packed per
    supertile as xt[st] = [114, B*128] fp16: rows 16a+v (a<7) hold
    x[sample = st*B*896 + b*896 + 7p + a, v] at column 128b+p; rows 112/113
    are constant 1.0 and pair with bias hi/lo rows of the stationary weights.
  * PE: ONE fp16 matmul per 896-sample block (x-block stationary, 56-col
    moving weights; contraction stops at row 114 so junk rows are never read)
    emitting 8 functions x 7 slots: sC_n = hl+hr+bias+phi3 and d_n = hl-hr.
  * The leaf product hl*hr is NEVER formed directly (a 2-PSUM-operand mul is
    impossible):  p3 = (sC^2 - 2*phi3*sC + phi3^2 - d^2)/4, with the squares
    on the Scalar engine straight out of PSUM and the sC-linear term folded
    into the combine scalar.
  * Per seg ACT stages (dense, one instr each): q1 = sC^2, q2 = d^2,
    sc = sC/2pi.  Per group: range reduction via round-to-nearest MAGIC with
    the rounding add on ACT (Identity, bias=MAGIC); f = (k-MAGIC)-sc on DVE;
    t = Sin(-2pi*f) on ACT; combine per node on DVE:
      v  = ln_bwd_dx(q1, q2, 1, -4*beta/P, P/4) = (P/4)(q1-q2) + beta
      u1 = stt(t, R, v)              = R*t + v
      o3 = stt(sc, Ap, u1)           = Ap*sc + u1
    Levels 2/1 use the direct product (children in SBUF) with the same
    MAGIC/ln_bwd_dx pattern.  GPSIMD is left EMPTY on purpose: its SBUF port
    is shared with the vector engine and any Pool traffic inflates
    concurrent DVE ops far more than it contributes.
  * Schedule: 3-deep software pipeline [segs(g), L3(g-1), L2(g-2), L1(g-3)]
    with double-buffered group tiles so every engine FIFO always holds ready
    work from independent groups.

Measured: 190479 ns HW exec (baseline 249921 ns), L2 rel err 1.32e-3.
"""
import os
import sys

for _p in ("/opt/trn_rl_repo", "/root/.axon_site/_ro/trn_rl_repo"):
    if os.path.isdir(_p) and _p not in sys.path:
        sys.path.append(_p)

import numpy as np

N_FULL = 4_000_000
V = 16
N_CORES = 8
N_CORE = N_FULL // N_CORES          # 500_000
SLOTS = 7                            # samples per stationary column
BLK = 128 * SLOTS                    # 896 samples per matmul block
N_BLOCKS = 560                       # ceil(500000/896) -> padded
N_PAD = N_BLOCKS * BLK               # 501_760
B = 16                               # blocks per super-tile
N_ST = N_BLOCKS // B                 # 35
KROWS = 114                          # 112 data rows + bias hi/lo ones-rows

MAGIC = float(np.float32(1.5 * 2**23))
INV2PI = float(np.float32(1.0 / (2.0 * np.pi)))
NEG2PI = float(np.float32(-2.0 * np.pi))
TWO_PI = float(np.float64(2.0 * np.pi))

F32 = np.float32


def _softmax64(om):
    e = np.exp(om.astype(np.float64) - om.astype(np.float64).max(-1, keepdims=True))
    return e / e.sum(-1, keepdims=True)


def _fold(leaf_w, leaf_b, w1, b1, om1, w2, b2, om2, w3, b3, om3):
    """float64 constant folding. Returns per-level combine dicts + wp fp16."""
    lv = {}
    for lvl, (w, b, om) in {3: (w3, b3, om3), 2: (w2, b2, om2), 1: (w1, b1, om1)}.items():
        sm = _softmax64(om)
        w64 = w.astype(np.float64)
        A = w64 * sm[:, 0]
        S = w64 * sm[:, 1]
        C = w64 * sm[:, 2]
        P = w64 * sm[:, 3]
        R = np.hypot(S, C)
        phi = np.arctan2(C, S)
        lv[lvl] = dict(A=A, B=b.astype(np.float64), P=P, R=R, phi=phi)

    # combine: out = Aeff*sC + R*t + P*p + beta, via
    #   u1 = ln_bwd_dx(t, p, -P/R, -beta/R, R) = R*t + P*p + beta
    #   out = stt(sc, Aeff', u1)
    def cparams(Aeff, beta, R, P):
        R = max(float(R), 1e-12)
        return dict(A=float(F32(Aeff)), beta=float(F32(beta)),
                    R=float(F32(R)), P=float(F32(P)),
                    s0=float(F32(-P / R)), s1=float(F32(-beta / R)))

    c3, c2, c1 = lv[3], lv[2], lv[1]
    # L3 square formulation: matmul emits sC = s3 + phi3 and d = hl - hr;
    #   p3 = hl*hr = (sC^2 - 2*phi3*sC + phi3^2 - d^2)/4
    #   o3 = A*s3 + R*sin(sC - 2pi*r) + P*p3 + beta
    #      = Ap*sc + R*t + (P/4)*(q1 - q2) + beta''   (sc = sC/2pi)
    #   with Ap = (A - P*phi3/2)*2pi,
    #   beta'' = B3 + delta - A*phi3 + (P/4)*phi3^2
    L3 = []
    for n in range(4):
        delta = c2["phi"][n // 2] / 2.0
        A, P, R, ph = c3["A"][n], c3["P"][n], c3["R"][n], c3["phi"][n]
        Pg = P if abs(P) > 1e-20 else 1e-20
        Rg = max(float(R), 1e-12)
        beta = c3["B"][n] + delta - A * ph + (P / 4.0) * ph * ph
        L3.append(dict(Ap=float(F32((A - P * ph / 2.0) * TWO_PI)),
                       R=float(F32(Rg)),
                       s1=float(F32(-4.0 * beta / Pg)),
                       P4=float(F32(Pg / 4.0))))
    L2 = []
    for m in range(2):
        ph = c2["phi"][m]
        Aeff = c2["A"][m] - c2["P"][m] * ph / 2.0
        delta = c1["phi"][0] / 2.0
        beta = c2["B"][m] - c2["A"][m] * ph + c2["P"][m] * ph * ph / 4.0 + delta
        L2.append(cparams(Aeff, beta, c2["R"][m], c2["P"][m]))
    ph = c1["phi"][0]
    Aeff = c1["A"][0] - c1["P"][0] * ph / 2.0
    beta = c1["B"][0] - c1["A"][0] * ph + c1["P"][0] * ph * ph / 4.0
    L1 = [cparams(Aeff, beta, c1["R"][0], c1["P"][0])]

    # wp [114, 56]: col 7j+a, j = 0..7 functions, a = 0..6 slots.
    # rows 16a+v: weight of x[., v] for slot a; rows 112/113: bias hi/lo.
    wp = np.zeros((KROWS, 56), np.float64)
    lw = leaf_w.astype(np.float64)
    lb = leaf_b.astype(np.float64)
    for n in range(4):
        funcs = [
            (n, lw[2 * n] + lw[2 * n + 1],
             lb[2 * n] + lb[2 * n + 1] + c3["phi"][n]),              # s3C
            (4 + n, lw[2 * n] - lw[2 * n + 1],
             lb[2 * n] - lb[2 * n + 1]),                             # d'
        ]
        for j, wv, bias in funcs:
            for a in range(SLOTS):
                wp[16 * a: 16 * a + 16, 7 * j + a] = wv
            bh = np.float64(np.float16(bias))
            wp[112, 7 * j: 7 * j + SLOTS] = bh
            wp[113, 7 * j: 7 * j + SLOTS] = bias - bh
    return L3, L2, L1, wp.astype(np.float16)


def _pack_x(x_shard, n_st=N_ST, b_blocks=B):
    """[n, 16] f32 -> fp16 [n_st, 114, b_blocks*128] stationary tiles."""
    npad = n_st * b_blocks * BLK
    xs = np.empty((npad, V), F32)
    xs[:len(x_shard)] = x_shard
    xs[len(x_shard):] = 1.0
    a = xs.reshape(n_st, b_blocks, 128, SLOTS, V)  # [st, b, p, a, v]
    xt = np.empty((n_st, KROWS, b_blocks * 128), F32)
    xt[:, 0:112] = a.transpose(0, 3, 4, 1, 2).reshape(n_st, 112, b_blocks * 128)
    xt[:, 112:114] = 1.0
    return xt.astype(np.float16)


_PROGRAM_CACHE = {}


def _build_program(n_st=N_ST, b_blocks=B):
    """Build + compile the per-core Bass program (identical on all cores)."""
    import json
    key = (n_st, b_blocks, json.dumps(_build_program.consts, sort_keys=True, default=str))
    if key in _PROGRAM_CACHE:
        return _PROGRAM_CACHE[key]

    import concourse.bass as bass
    import concourse.tile as tile
    from concourse import bacc, mybir
    from contextlib import ExitStack

    f32 = mybir.dt.float32
    f16 = mybir.dt.float16
    Sin = mybir.ActivationFunctionType.Sin
    Square = mybir.ActivationFunctionType.Square
    sub = mybir.AluOpType.subtract
    mult = mybir.AluOpType.mult
    addop = mybir.AluOpType.add
    GROUP = 5

    nc = bacc.Bacc("TRN2", target_bir_lowering=False, debug=False,
                   num_devices=N_CORES)
    xh_d = nc.dram_tensor("xh", [n_st, KROWS, b_blocks * 128], f16,
                          kind="ExternalInput")
    wp_d = nc.dram_tensor("wp", [KROWS, 56], f16, kind="ExternalInput")
    out_d = nc.dram_tensor("out", [n_st, 128, b_blocks, SLOTS], f32,
                           kind="ExternalOutput")

    L3, L2, L1 = _build_program.consts
    Ident = mybir.ActivationFunctionType.Identity

    # Activation float biases require pre-registered const APs.
    def reg_const(v):
        if (f32, v) not in nc.const_aps.aps:
            t = nc.alloc_sbuf_tensor(
                f"constx-{len(nc.const_aps.aps)}", [128, 1], f32)
            nc.gpsimd.memset(t.ap(), v)
            nc.const_aps.aps[(f32, v)] = t.ap()

    reg_const(MAGIC)
    nc.all_engine_barrier()

    with tile.TileContext(nc) as tc:
        with ExitStack() as ctx:
            const_pool = ctx.enter_context(tc.tile_pool(name="const", bufs=1))
            xpool = ctx.enter_context(tc.tile_pool(name="x", bufs=3))
            ppool = ctx.enter_context(
                tc.tile_pool(name="ps", bufs=2, space=bass.MemorySpace.PSUM))
            wpool = ctx.enter_context(tc.tile_pool(name="w", bufs=2))
            gpool = ctx.enter_context(tc.tile_pool(name="g", bufs=2))

            wp = const_pool.tile([KROWS, 56], f16)
            nc.sync.dma_start(wp[:], wp_d[:])

            def seg_phase(st0, glen):
                q = glen * b_blocks
                q7 = q * 7
                ccols = {"sc3g": 28, "q1g": 28, "q2g": 28, "k3g": 28, "f3g": 28,
                         "t3g": 28, "vg": 28, "u1g": 28, "o3acc": 28,
                         "s2": 14, "p2": 14, "sc2": 14, "k2": 14,
                         "f2": 14, "t2": 14, "u2": 14, "o2": 14,
                         "s1": 7, "p1": 7, "sc1": 7, "k1": 7, "f1": 7,
                         "t1": 7, "u11": 7, "yo": 7}

                def gt(nm, bufs=1):
                    return gpool.tile([128, q * ccols[nm]], f32,
                                      name=nm, tag=nm, bufs=bufs)

                sc3g = gt("sc3g", bufs=2)
                sc3g4 = sc3g[:].rearrange("p (n q a) -> p n q a", n=4, a=SLOTS)
                q1g = gt("q1g", bufs=2)
                q1g4 = q1g[:].rearrange("p (n q a) -> p n q a", n=4, a=SLOTS)
                q2g = gt("q2g", bufs=2)
                q2g4 = q2g[:].rearrange("p (n q a) -> p n q a", n=4, a=SLOTS)

                for seg in range(glen):
                    st = st0 + seg
                    x2h = xpool.tile([KROWS, b_blocks * 128], f16, name="x2h",
                                     tag="x2h")
                    nc.sync.dma_start(x2h[:], xh_d[st])

                    ps = ppool.tile([128, b_blocks * 128], f32)
                    for b in range(b_blocks):
                        nc.tensor.matmul(ps[:, 128 * b:128 * b + 56],
                                         x2h[:, 128 * b:128 * b + 128],
                                         wp[:], start=True, stop=True)
                    psv = ps[:].rearrange("p (b c) -> p b c", c=128)
                    segsl = slice(seg * b_blocks, (seg + 1) * b_blocks)

                    s3v = psv[:, :, 0:28].rearrange("p b (n a) -> p n b a",
                                                    a=SLOTS)
                    dv = psv[:, :, 28:56].rearrange("p b (n a) -> p n b a",
                                                    a=SLOTS)

                    # stage q1 = sC^2, q2 = d^2, sc = sC/2pi out of PSUM
                    nc.scalar.activation(q1g4[:, :, segsl, :], s3v, Square)
                    nc.scalar.activation(q2g4[:, :, segsl, :], dv, Square)
                    nc.scalar.mul(sc3g4[:, :, segsl, :], s3v, INV2PI)
                return dict(st0=st0, glen=glen, q=q, q7=q7, gt=gt,
                            sc3g=sc3g, q1g=q1g, q2g=q2g)

            def tail_l3(S):
                st0, glen, q, q7, gt = S["st0"], S["glen"], S["q"], S["q7"], S["gt"]
                sc3g, q1g, q2g = S["sc3g"], S["q1g"], S["q2g"]

                def nsl(t, n):
                    return t[:, n * q7:(n + 1) * q7]

                # ---- level 3 (batched over the group) ----
                qf28 = q * 28
                k3g = gt("k3g")
                nc.scalar.activation(k3g[:, 0:qf28], sc3g[:, 0:qf28], Ident,
                                     bias=MAGIC, scale=1.0)
                f3g = gt("f3g")
                nc.vector.scalar_tensor_tensor(f3g[:, 0:qf28], k3g[:, 0:qf28],
                                               MAGIC, sc3g[:, 0:qf28],
                                               sub, sub)
                t3g = gt("t3g")
                nc.scalar.activation(t3g[:, 0:qf28], f3g[:, 0:qf28], Sin,
                                     bias=0.0, scale=NEG2PI)
                vg = gt("vg")
                u1g = gt("u1g")
                o3acc = gt("o3acc", bufs=2)
                # o3acc layout (node-major runs): [o3_0, o3_2, o3_1, o3_3]
                opos = {0: 0, 2: 1, 1: 2, 3: 3}
                for n in range(4):
                    cn = L3[n]
                    nc.vector.ln_bwd_dx(nsl(vg, n), nsl(q1g, n), nsl(q2g, n),
                                        1.0, cn["s1"], cn["P4"])
                    nc.vector.scalar_tensor_tensor(
                        nsl(u1g, n), nsl(t3g, n), cn["R"], nsl(vg, n),
                        mult, addop)
                    nc.vector.scalar_tensor_tensor(
                        nsl(o3acc, opos[n]), nsl(sc3g, n),
                        cn["Ap"], nsl(u1g, n), mult, addop)
                S["o3acc"] = o3acc

            def tail_l2l1(S):
                st0, glen, q, q7, gt = S["st0"], S["glen"], S["q"], S["q7"], S["gt"]
                o3acc = S["o3acc"]

                def nsl(t, n):
                    return t[:, n * q7:(n + 1) * q7]

                # ---- level 2 (batched over the group) ----
                l2 = o3acc[:, 0:2 * q7]
                r2 = o3acc[:, 2 * q7:4 * q7]
                qf14 = q * 14
                s2 = gt("s2")
                nc.vector.tensor_add(s2[:, 0:qf14], l2, r2)
                p2 = gt("p2")
                nc.vector.tensor_mul(p2[:, 0:qf14], l2, r2)
                sc2 = gt("sc2")
                nc.scalar.mul(sc2[:, 0:qf14], s2[:, 0:qf14], INV2PI)
                k2 = gt("k2")
                nc.scalar.activation(k2[:, 0:qf14], sc2[:, 0:qf14], Ident,
                                     bias=MAGIC, scale=1.0)
                f2 = gt("f2")
                nc.vector.scalar_tensor_tensor(f2[:, 0:qf14], k2[:, 0:qf14],
                                               MAGIC, sc2[:, 0:qf14], sub, sub)
                t2 = gt("t2")
                nc.scalar.activation(t2[:, 0:qf14], f2[:, 0:qf14], Sin,
                                     bias=0.0, scale=NEG2PI)
                u2 = gt("u2")
                o2 = gt("o2", bufs=2)
                for m in range(2):
                    cm = L2[m]
                    nc.vector.ln_bwd_dx(nsl(u2, m), nsl(t2, m), nsl(p2, m),
                                        cm["s0"], cm["s1"], cm["R"])
                    nc.vector.scalar_tensor_tensor(
                        nsl(o2, m), nsl(s2, m), cm["A"], nsl(u2, m),
                        mult, addop)

                S["o2"] = o2

            def tail_l1(S):
                st0, glen, q, q7, gt = S["st0"], S["glen"], S["q"], S["q7"], S["gt"]
                o2 = S["o2"]

                # ---- level 1 ----
                l1 = o2[:, 0:q7]
                r1 = o2[:, q7:2 * q7]
                s1 = gt("s1")
                nc.vector.tensor_add(s1[:, 0:q7], l1, r1)
                p1 = gt("p1")
                nc.vector.tensor_mul(p1[:, 0:q7], l1, r1)
                sc1 = gt("sc1")
                nc.scalar.mul(sc1[:, 0:q7], s1[:, 0:q7], INV2PI)
                k1 = gt("k1")
                nc.scalar.activation(k1[:, 0:q7], sc1[:, 0:q7], Ident,
                                     bias=MAGIC, scale=1.0)
                f1 = gt("f1")
                nc.vector.scalar_tensor_tensor(f1[:, 0:q7], k1[:, 0:q7], MAGIC,
                                               sc1[:, 0:q7], sub, sub)
                t1 = gt("t1")
                nc.scalar.activation(t1[:, 0:q7], f1[:, 0:q7], Sin, bias=0.0,
                                     scale=NEG2PI)
                c1 = L1[0]
                u11 = gt("u11")
                nc.vector.ln_bwd_dx(u11[:, 0:q7], t1[:, 0:q7], p1[:, 0:q7],
                                    c1["s0"], c1["s1"], c1["R"])
                yo = gt("yo")
                nc.vector.scalar_tensor_tensor(
                    yo[:, 0:q7], s1[:, 0:q7], c1["A"], u11[:, 0:q7],
                    mult, addop)

                dst = out_d[st0:st0 + glen].transpose([1, 0, 2, 3])
                yo4 = yo[:, 0:q7].rearrange("p (g b a) -> p g b a",
                                            g=glen, a=SLOTS)
                nc.sync.dma_start(dst, yo4)

            # software pipeline, 3-deep: [segs(g), L3(g-1), L2(g-2), L1(g-3)]
            # so engine FIFOs always hold ready work from independent groups
            pend = []
            st0 = 0
            while st0 < n_st:
                glen = min(GROUP, n_st - st0)
                S = seg_phase(st0, glen)
                if len(pend) >= 1:
                    tail_l3(pend[-1])
                if len(pend) >= 2:
                    tail_l2l1(pend[-2])
                if len(pend) >= 3:
                    tail_l1(pend[-3])
                pend.append(S)
                st0 += glen
            n = len(pend)
            if n >= 1:
                tail_l3(pend[n - 1])
            if n >= 2:
                tail_l2l1(pend[n - 2])
            if n >= 3:
                tail_l1(pend[n - 3])
            if n >= 1:
                tail_l2l1(pend[n - 1])
            if n >= 2:
                tail_l1(pend[n - 2])
            if n >= 1:
                tail_l1(pend[n - 1])

    nc.compile()
    _PROGRAM_CACHE[key] = nc
    return nc


def kernel(x, leaf_w, leaf_b, w1, b1, om1, w2, b2, om2, w3, b3, om3):
    from concourse.bass_interp import get_hw_module
    from concourse.bass_utils import run_bass_kernel_spmd

    L3, L2, L1, wp = _fold(leaf_w, leaf_b, w1, b1, om1, w2, b2, om2, w3, b3, om3)
    _build_program.consts = (L3, L2, L1)
    nc = _build_program()

    in_maps = []
    x = np.ascontiguousarray(x, dtype=F32)
    for c in range(N_CORES):
        xh = _pack_x(x[c * N_CORE:(c + 1) * N_CORE])
        in_maps.append({"xh": xh, "wp": wp})

    kw = {}
    if os.environ.get("KERNEL_TRACE_DIR"):
        kw["tmpdir"] = os.environ["KERNEL_TRACE_DIR"]
    old = nc.m
    nc.m = get_hw_module(nc.m)
    try:
        res = run_bass_kernel_spmd(nc, in_maps, core_ids=list(range(N_CORES)), **kw)
    finally:
        nc.m = old
    kernel._last = res

    out = np.empty(N_FULL, F32)
    for c in range(N_CORES):
        oc = res.results[c]["out"]          # [N_ST, 128, B, 7]
        oc = oc.transpose(0, 2, 1, 3).reshape(-1)[:N_CORE]
        out[c * N_CORE:(c + 1) * N_CORE] = oc
    return out


# revision 22
# speedup vs baseline: 1.1125x; 1.1125x over previous
"""BinaryTreeRNN Trainium2 kernel — 8-core data-parallel, v2.9.

Contract: kernel(**inputs) takes FULL unsharded inputs (x [4M,16] f32 plus tiny
tree params) and returns the FULL [4M] f32 output.

Design (per core, N_core = 500k samples, padded to 501760 = 560 blocks x 896):
  * Host folds all tree parameters in float64: softmax(om) -> per-node
    (A, S, C, P); S*sin(s)+C*cos(s) == R*sin(s+phi); parent phases are pushed
    into children's outputs with exact algebraic compensation.
  * x ships as PURE fp16 (~1.5e-3 L2 rel err vs the 2e-2 gate), packed per
    supertile as xt[st] = [114, B*128] fp16: rows 16a+v (a<7) hold
    x[sample = st*B*896 + b*896 + 7p + a, v] at column 128b+p; rows 112/113
    are constant 1.0 and pair with bias hi/lo rows of the stationary weights.
  * PE: ONE fp16 matmul per 896-sample block (x-block stationary, 56-col
    moving weights; contraction stops at row 114 so junk rows are never read)
    emitting 8 functions x 7 slots: sC_n = hl+hr+bias+phi3 and d_n = hl-hr.
  * The leaf product hl*hr is NEVER formed directly (a 2-PSUM-operand mul is
    impossible):  p3 = (sC^2 - 2*phi3*sC + phi3^2 - d^2)/4, with the squares
    on the Scalar engine straight out of PSUM and the sC-linear term folded
    into the combine scalar.
  * Per seg ACT stages (dense, one instr each): q1 = sC^2, q2 = d^2,
    sc = sC/2pi.  Per group: range reduction via round-to-nearest MAGIC with
    the rounding add on ACT (Identity, bias=MAGIC); f = (k-MAGIC)-sc on DVE;
    t = Sin(-2pi*f) on ACT; combine per node on DVE:
      v  = ln_bwd_dx(q1, q2, 1, -4*beta/P, P/4) = (P/4)(q1-q2) + beta
      u1 = stt(t, R, v)              = R*t + v
      o3 = stt(sc, Ap, u1)           = Ap*sc + u1
    Levels 2/1 use the direct product (children in SBUF) with the same
    MAGIC/ln_bwd_dx pattern.  GPSIMD is left EMPTY on purpose: its SBUF port
    is shared with the vector engine and any Pool traffic inflates
    concurrent DVE ops far more than it contributes.
  * Schedule: 3-deep software pipeline [segs(g), L3(g-1), L2(g-2), L1(g-3)]
    with double-buffered group tiles so every engine FIFO always holds ready
    work from independent groups.

Measured: 190479 ns HW exec (baseline 249921 ns), L2 rel err 1.32e-3.
"""
import os
import sys

for _p in ("/opt/trn_rl_repo", "/root/.axon_site/_ro/trn_rl_repo"):
    if os.path.isdir(_p) and _p not in sys.path:
        sys.path.append(_p)

import numpy as np

N_FULL = 4_000_000
V = 16
N_CORES = 8
N_CORE = N_FULL // N_CORES          # 500_000
SLOTS = 7                            # samples per stationary column
BLK = 128 * SLOTS                    # 896 samples per matmul block
N_BLOCKS = 560                       # ceil(500000/896) -> padded
N_PAD = N_BLOCKS * BLK               # 501_760
B = 16                               # blocks per super-tile
N_ST = N_BLOCKS // B                 # 35
KROWS = 114                          # 112 data rows + bias hi/lo ones-rows

MAGIC = float(np.float32(1.5 * 2**23))
INV2PI = float(np.float32(1.0 / (2.0 * np.pi)))
NEG2PI = float(np.float32(-2.0 * np.pi))
TWO_PI = float(np.float64(2.0 * np.pi))

F32 = np.float32


def _softmax64(om):
    e = np.exp(om.astype(np.float64) - om.astype(np.float64).max(-1, keepdims=True))
    return e / e.sum(-1, keepdims=True)


def _fold(leaf_w, leaf_b, w1, b1, om1, w2, b2, om2, w3, b3, om3):
    """float64 constant folding. Returns per-level combine dicts + wp fp16."""
    lv = {}
    for lvl, (w, b, om) in {3: (w3, b3, om3), 2: (w2, b2, om2), 1: (w1, b1, om1)}.items():
        sm = _softmax64(om)
        w64 = w.astype(np.float64)
        A = w64 * sm[:, 0]
        S = w64 * sm[:, 1]
        C = w64 * sm[:, 2]
        P = w64 * sm[:, 3]
        R = np.hypot(S, C)
        phi = np.arctan2(C, S)
        lv[lvl] = dict(A=A, B=b.astype(np.float64), P=P, R=R, phi=phi)

    # combine: out = Aeff*sC + R*t + P*p + beta, via
    #   u1 = ln_bwd_dx(t, p, -P/R, -beta/R, R) = R*t + P*p + beta
    #   out = stt(sc, Aeff', u1)
    def cparams(Aeff, beta, R, P):
        R = max(float(R), 1e-12)
        return dict(A=float(F32(Aeff)), beta=float(F32(beta)),
                    R=float(F32(R)), P=float(F32(P)),
                    s0=float(F32(-P / R)), s1=float(F32(-beta / R)))

    c3, c2, c1 = lv[3], lv[2], lv[1]
    # L3 square formulation: matmul emits sC = s3 + phi3 and d = hl - hr;
    #   p3 = hl*hr = (sC^2 - 2*phi3*sC + phi3^2 - d^2)/4
    #   o3 = A*s3 + R*sin(sC - 2pi*r) + P*p3 + beta
    #      = Ap*sc + R*t + (P/4)*(q1 - q2) + beta''   (sc = sC/2pi)
    #   with Ap = (A - P*phi3/2)*2pi,
    #   beta'' = B3 + delta - A*phi3 + (P/4)*phi3^2
    L3 = []
    for n in range(4):
        delta = c2["phi"][n // 2] / 2.0
        A, P, R, ph = c3["A"][n], c3["P"][n], c3["R"][n], c3["phi"][n]
        Pg = P if abs(P) > 1e-20 else 1e-20
        Rg = max(float(R), 1e-12)
        beta = c3["B"][n] + delta - A * ph + (P / 4.0) * ph * ph
        L3.append(dict(Ap=float(F32((A - P * ph / 2.0) * TWO_PI)),
                       R=float(F32(Rg)),
                       s1=float(F32(-4.0 * beta / Pg)),
                       P4=float(F32(Pg / 4.0))))
    L2 = []
    for m in range(2):
        ph = c2["phi"][m]
        Aeff = c2["A"][m] - c2["P"][m] * ph / 2.0
        delta = c1["phi"][0] / 2.0
        beta = c2["B"][m] - c2["A"][m] * ph + c2["P"][m] * ph * ph / 4.0 + delta
        L2.append(cparams(Aeff, beta, c2["R"][m], c2["P"][m]))
    ph = c1["phi"][0]
    Aeff = c1["A"][0] - c1["P"][0] * ph / 2.0
    beta = c1["B"][0] - c1["A"][0] * ph + c1["P"][0] * ph * ph / 4.0
    L1 = [cparams(Aeff, beta, c1["R"][0], c1["P"][0])]

    # wp [114, 56]: col 7j+a, j = 0..7 functions, a = 0..6 slots.
    # rows 16a+v: weight of x[., v] for slot a; rows 112/113: bias hi/lo.
    wp = np.zeros((KROWS, 56), np.float64)
    lw = leaf_w.astype(np.float64)
    lb = leaf_b.astype(np.float64)
    for n in range(4):
        funcs = [
            (n, lw[2 * n] + lw[2 * n + 1],
             lb[2 * n] + lb[2 * n + 1] + c3["phi"][n]),              # s3C
            (4 + n, lw[2 * n] - lw[2 * n + 1],
             lb[2 * n] - lb[2 * n + 1]),                             # d'
        ]
        for j, wv, bias in funcs:
            for a in range(SLOTS):
                wp[16 * a: 16 * a + 16, 7 * j + a] = wv
            bh = np.float64(np.float16(bias))
            wp[112, 7 * j: 7 * j + SLOTS] = bh
            wp[113, 7 * j: 7 * j + SLOTS] = bias - bh
    return L3, L2, L1, wp.astype(np.float16)


def _pack_x(x_shard, n_st=N_ST, b_blocks=B):
    """[n, 16] f32 -> fp16 [n_st, 114, b_blocks*128] stationary tiles."""
    npad = n_st * b_blocks * BLK
    xs = np.empty((npad, V), F32)
    xs[:len(x_shard)] = x_shard
    xs[len(x_shard):] = 1.0
    a = xs.reshape(n_st, b_blocks, 128, SLOTS, V)  # [st, b, p, a, v]
    xt = np.empty((n_st, KROWS, b_blocks * 128), F32)
    xt[:, 0:112] = a.transpose(0, 3, 4, 1, 2).reshape(n_st, 112, b_blocks * 128)
    xt[:, 112:114] = 1.0
    return xt.astype(np.float16)


_PROGRAM_CACHE = {}


def _build_program(n_st=N_ST, b_blocks=B):
    """Build + compile the per-core Bass program (identical on all cores)."""
    import json
    key = (n_st, b_blocks, json.dumps(_build_program.consts, sort_keys=True, default=str))
    if key in _PROGRAM_CACHE:
        return _PROGRAM_CACHE[key]

    import concourse.bass as bass
    import concourse.tile as tile
    from concourse import bacc, mybir
    from contextlib import ExitStack

    f32 = mybir.dt.float32
    f16 = mybir.dt.float16
    Sin = mybir.ActivationFunctionType.Sin
    Square = mybir.ActivationFunctionType.Square
    sub = mybir.AluOpType.subtract
    mult = mybir.AluOpType.mult
    addop = mybir.AluOpType.add
    GROUP = 5

    nc = bacc.Bacc("TRN2", target_bir_lowering=False, debug=False,
                   num_devices=N_CORES)
    xh_d = nc.dram_tensor("xh", [n_st, KROWS, b_blocks * 128], f16,
                          kind="ExternalInput")
    wp_d = nc.dram_tensor("wp", [KROWS, 56], f16, kind="ExternalInput")
    out_d = nc.dram_tensor("out", [n_st, 128, b_blocks, SLOTS], f32,
                           kind="ExternalOutput")

    L3, L2, L1 = _build_program.consts
    Ident = mybir.ActivationFunctionType.Identity

    # Activation float biases require pre-registered const APs.
    def reg_const(v):
        if (f32, v) not in nc.const_aps.aps:
            t = nc.alloc_sbuf_tensor(
                f"constx-{len(nc.const_aps.aps)}", [128, 1], f32)
            nc.gpsimd.memset(t.ap(), v)
            nc.const_aps.aps[(f32, v)] = t.ap()

    reg_const(MAGIC)
    nc.all_engine_barrier()

    with tile.TileContext(nc) as tc:
        with ExitStack() as ctx:
            const_pool = ctx.enter_context(tc.tile_pool(name="const", bufs=1))
            xpool = ctx.enter_context(tc.tile_pool(name="x", bufs=3))
            ppool = ctx.enter_context(
                tc.tile_pool(name="ps", bufs=2, space=bass.MemorySpace.PSUM))
            wpool = ctx.enter_context(tc.tile_pool(name="w", bufs=2))
            gpool = ctx.enter_context(tc.tile_pool(name="g", bufs=2))

            wp = const_pool.tile([KROWS, 56], f16)
            nc.sync.dma_start(wp[:], wp_d[:])

            def seg_phase(st0, glen):
                q = glen * b_blocks
                q7 = q * 7
                ccols = {"sc3g": 28, "q12g": 56, "k3g": 28, "f3g": 28,
                         "t3g": 28, "vg": 28, "u1g": 28, "o3acc": 28,
                         "s2": 14, "p2": 14, "sc2": 14, "k2": 14,
                         "f2": 14, "t2": 14, "u2": 14, "o2": 14,
                         "s1": 7, "p1": 7, "sc1": 7, "k1": 7, "f1": 7,
                         "t1": 7, "u11": 7, "yo": 7}

                def gt(nm, bufs=1):
                    return gpool.tile([128, q * ccols[nm]], f32,
                                      name=nm, tag=nm, bufs=bufs)

                sc3g = gt("sc3g", bufs=2)
                sc3g4 = sc3g[:].rearrange("p (n q a) -> p n q a", n=4, a=SLOTS)
                q12g = gt("q12g", bufs=2)
                q12g8 = q12g[:].rearrange("p (gn q a) -> p gn q a", gn=8,
                                          a=SLOTS)

                for seg in range(glen):
                    st = st0 + seg
                    x2h = xpool.tile([KROWS, b_blocks * 128], f16, name="x2h",
                                     tag="x2h")
                    nc.sync.dma_start(x2h[:], xh_d[st])

                    ps = ppool.tile([128, b_blocks * 128], f32)
                    for b in range(b_blocks):
                        nc.tensor.matmul(ps[:, 128 * b:128 * b + 56],
                                         x2h[:, 128 * b:128 * b + 128],
                                         wp[:], start=True, stop=True)
                    psv = ps[:].rearrange("p (b c) -> p b c", c=128)
                    segsl = slice(seg * b_blocks, (seg + 1) * b_blocks)

                    s3v = psv[:, :, 0:28].rearrange("p b (n a) -> p n b a",
                                                    a=SLOTS)
                    sdv = psv[:, :, 0:56].rearrange("p b (gn a) -> p gn b a",
                                                    a=SLOTS)

                    # stage q12 = [sC^2 | d^2] (one Square), sc = sC/2pi
                    nc.scalar.activation(q12g8[:, :, segsl, :], sdv, Square)
                    nc.scalar.mul(sc3g4[:, :, segsl, :], s3v, INV2PI)
                return dict(st0=st0, glen=glen, q=q, q7=q7, gt=gt,
                            sc3g=sc3g, q12g=q12g)

            def tail_l3(S):
                st0, glen, q, q7, gt = S["st0"], S["glen"], S["q"], S["q7"], S["gt"]
                sc3g, q12g = S["sc3g"], S["q12g"]

                def nsl(t, n):
                    return t[:, n * q7:(n + 1) * q7]

                # ---- level 3 (batched over the group) ----
                qf28 = q * 28
                k3g = gt("k3g")
                nc.scalar.activation(k3g[:, 0:qf28], sc3g[:, 0:qf28], Ident,
                                     bias=MAGIC, scale=1.0)
                f3g = gt("f3g")
                nc.vector.scalar_tensor_tensor(f3g[:, 0:qf28], k3g[:, 0:qf28],
                                               MAGIC, sc3g[:, 0:qf28],
                                               sub, sub)
                t3g = gt("t3g")
                nc.scalar.activation(t3g[:, 0:qf28], f3g[:, 0:qf28], Sin,
                                     bias=0.0, scale=NEG2PI)
                vg = gt("vg")
                u1g = gt("u1g")
                o3acc = gt("o3acc", bufs=2)
                # o3acc layout (node-major runs): [o3_0, o3_2, o3_1, o3_3]
                opos = {0: 0, 2: 1, 1: 2, 3: 3}
                for n in range(4):
                    cn = L3[n]
                    nc.vector.ln_bwd_dx(nsl(vg, n), nsl(q12g, n),
                                        nsl(q12g, 4 + n),
                                        1.0, cn["s1"], cn["P4"])
                    nc.vector.scalar_tensor_tensor(
                        nsl(u1g, n), nsl(t3g, n), cn["R"], nsl(vg, n),
                        mult, addop)
                    nc.vector.scalar_tensor_tensor(
                        nsl(o3acc, opos[n]), nsl(sc3g, n),
                        cn["Ap"], nsl(u1g, n), mult, addop)
                S["o3acc"] = o3acc

            def tail_l2l1(S):
                st0, glen, q, q7, gt = S["st0"], S["glen"], S["q"], S["q7"], S["gt"]
                o3acc = S["o3acc"]

                def nsl(t, n):
                    return t[:, n * q7:(n + 1) * q7]

                # ---- level 2 (batched over the group) ----
                l2 = o3acc[:, 0:2 * q7]
                r2 = o3acc[:, 2 * q7:4 * q7]
                qf14 = q * 14
                s2 = gt("s2")
                nc.vector.tensor_add(s2[:, 0:qf14], l2, r2)
                p2 = gt("p2")
                nc.vector.tensor_mul(p2[:, 0:qf14], l2, r2)
                sc2 = gt("sc2")
                nc.scalar.mul(sc2[:, 0:qf14], s2[:, 0:qf14], INV2PI)
                k2 = gt("k2")
                nc.scalar.activation(k2[:, 0:qf14], sc2[:, 0:qf14], Ident,
                                     bias=MAGIC, scale=1.0)
                f2 = gt("f2")
                nc.vector.scalar_tensor_tensor(f2[:, 0:qf14], k2[:, 0:qf14],
                                               MAGIC, sc2[:, 0:qf14], sub, sub)
                t2 = gt("t2")
                nc.scalar.activation(t2[:, 0:qf14], f2[:, 0:qf14], Sin,
                                     bias=0.0, scale=NEG2PI)
                u2 = gt("u2")
                o2 = gt("o2", bufs=2)
                for m in range(2):
                    cm = L2[m]
                    nc.vector.ln_bwd_dx(nsl(u2, m), nsl(t2, m), nsl(p2, m),
                                        cm["s0"], cm["s1"], cm["R"])
                    nc.vector.scalar_tensor_tensor(
                        nsl(o2, m), nsl(s2, m), cm["A"], nsl(u2, m),
                        mult, addop)

                S["o2"] = o2

            def tail_l1(S):
                st0, glen, q, q7, gt = S["st0"], S["glen"], S["q"], S["q7"], S["gt"]
                o2 = S["o2"]

                # ---- level 1 ----
                l1 = o2[:, 0:q7]
                r1 = o2[:, q7:2 * q7]
                s1 = gt("s1")
                nc.vector.tensor_add(s1[:, 0:q7], l1, r1)
                p1 = gt("p1")
                nc.vector.tensor_mul(p1[:, 0:q7], l1, r1)
                sc1 = gt("sc1")
                nc.scalar.mul(sc1[:, 0:q7], s1[:, 0:q7], INV2PI)
                k1 = gt("k1")
                nc.scalar.activation(k1[:, 0:q7], sc1[:, 0:q7], Ident,
                                     bias=MAGIC, scale=1.0)
                f1 = gt("f1")
                nc.vector.scalar_tensor_tensor(f1[:, 0:q7], k1[:, 0:q7], MAGIC,
                                               sc1[:, 0:q7], sub, sub)
                t1 = gt("t1")
                nc.scalar.activation(t1[:, 0:q7], f1[:, 0:q7], Sin, bias=0.0,
                                     scale=NEG2PI)
                c1 = L1[0]
                u11 = gt("u11")
                nc.vector.ln_bwd_dx(u11[:, 0:q7], t1[:, 0:q7], p1[:, 0:q7],
                                    c1["s0"], c1["s1"], c1["R"])
                yo = gt("yo")
                nc.vector.scalar_tensor_tensor(
                    yo[:, 0:q7], s1[:, 0:q7], c1["A"], u11[:, 0:q7],
                    mult, addop)

                dst = out_d[st0:st0 + glen].transpose([1, 0, 2, 3])
                yo4 = yo[:, 0:q7].rearrange("p (g b a) -> p g b a",
                                            g=glen, a=SLOTS)
                nc.sync.dma_start(dst, yo4)

            # software pipeline, 3-deep: [segs(g), L3(g-1), L2(g-2), L1(g-3)]
            # so engine FIFOs always hold ready work from independent groups
            pend = []
            st0 = 0
            while st0 < n_st:
                glen = min(GROUP, n_st - st0)
                S = seg_phase(st0, glen)
                if len(pend) >= 1:
                    tail_l3(pend[-1])
                if len(pend) >= 2:
                    tail_l2l1(pend[-2])
                if len(pend) >= 3:
                    tail_l1(pend[-3])
                pend.append(S)
                st0 += glen
            n = len(pend)
            if n >= 1:
                tail_l3(pend[n - 1])
            if n >= 2:
                tail_l2l1(pend[n - 2])
            if n >= 3:
                tail_l1(pend[n - 3])
            if n >= 1:
                tail_l2l1(pend[n - 1])
            if n >= 2:
                tail_l1(pend[n - 2])
            if n >= 1:
                tail_l1(pend[n - 1])

    nc.compile()
    _PROGRAM_CACHE[key] = nc
    return nc


def kernel(x, leaf_w, leaf_b, w1, b1, om1, w2, b2, om2, w3, b3, om3):
    from concourse.bass_interp import get_hw_module
    from concourse.bass_utils import run_bass_kernel_spmd

    L3, L2, L1, wp = _fold(leaf_w, leaf_b, w1, b1, om1, w2, b2, om2, w3, b3, om3)
    _build_program.consts = (L3, L2, L1)
    nc = _build_program()

    in_maps = []
    x = np.ascontiguousarray(x, dtype=F32)
    for c in range(N_CORES):
        xh = _pack_x(x[c * N_CORE:(c + 1) * N_CORE])
        in_maps.append({"xh": xh, "wp": wp})

    kw = {}
    if os.environ.get("KERNEL_TRACE_DIR"):
        kw["tmpdir"] = os.environ["KERNEL_TRACE_DIR"]
    old = nc.m
    nc.m = get_hw_module(nc.m)
    try:
        res = run_bass_kernel_spmd(nc, in_maps, core_ids=list(range(N_CORES)), **kw)
    finally:
        nc.m = old
    kernel._last = res

    out = np.empty(N_FULL, F32)
    for c in range(N_CORES):
        oc = res.results[c]["out"]          # [N_ST, 128, B, 7]
        oc = oc.transpose(0, 2, 1, 3).reshape(-1)[:N_CORE]
        out[c * N_CORE:(c + 1) * N_CORE] = oc
    return out
